# revision 35
# baseline (speedup 1.0000x reference)
"""Trainium2 Bass kernel for NT-Xent / SimCLR contrastive loss, v3.

Design (8 cores, data-parallel over rows of z = concat(z_i, z_j)):
  Host pre-normalizes z (L2 rows), scales by 8, transposes to feature-
  major [512, 8192], casts to fp8e4, and rotates by c*1024 columns per
  core so every core's own rows sit at columns [0, 1024).

  Device per core (pure matmul + exp pipeline; no transposes, casts, or
  cross-core reduction):
    - DMA the fp8 operand array (4.2 MB) in 16 chunk-major column
      chunks [128, 4, 512] (2 KB contiguous per partition -> 128
      descriptors per chunk; the DGE rings are descriptor-rate-bound
      for smaller runs), early chunks interleaved across the scalar
      HWDGE + gpsimd SWDGE rings.
    - Sweep (s-outer, t-inner): stationary = my row-block t (fp8
      DoubleRow, K=512 in 2 passes), moving = column chunk s of 2048 ->
      psum [128, 2048] = 64*sim; the ramp is split into pieces ordered
      breadth-first over row-blocks so the exp stream starts ~11 us in
      and only touches delivered columns.
    - ScalarE exp(0.03125 * psum) -> per-row partial sums of
      exp(2*sim[r, :]); most chunks route the exp through an SBUF bf16
      tile so the idle Vector engine does the row-sum off the critical
      path, keeping ScalarE's queue free of accumulator reads; 36
      partials in acc[128, 36].
  The ScalarE exp stream is the bottleneck and runs near-gap-free to
  the end; matmuls (~64 us) and DVE reductions (~52 us) hide behind
  it.
  Host: denom[r] = sum acc cols per t - exp(2); positives from fp32 z;
  loss = mean(log(denom) - 2*pos).
"""

import sys

if "/opt/trn_rl_repo" not in sys.path:
    sys.path.insert(0, "/opt/trn_rl_repo")

import numpy as np

N = 4096
D = 512
TEMP = 0.5
INV_T = 1.0 / TEMP

N2 = 2 * N            # 8192
NCORES = 8
ROWS = N2 // NCORES   # 1024 rows per core
P = 128
MT = ROWS // P        # 8 stationary row-blocks per core
SW = 2048             # moving sweep chunk (4 psum banks)
NSW = N2 // SW        # 4 sweep chunks
KP = 2                # DoubleRow K passes (256 features each)
SC = 8.0              # fp8 operand scale; psum = SC*SC*sim
# accumulator columns in device emission order: ramp pieces
# (t0/t0/t1/t0/t1), s=0 remainder, then s=1..3 with the last chunk split
ACC_T = (
    [0, 0, 0, 1, 0, 1]
    + list(range(2, MT))
    + [_t for _s in range(1, NSW) for _t in range(MT)]
    + [MT - 1]
)
ACOLS = len(ACC_T)   # 37

_CACHE = {}


def build(debug=False):
    import concourse.bacc as bacc
    import concourse.tile as tile
    from concourse import mybir

    f32 = mybir.dt.float32
    bf16 = mybir.dt.bfloat16
    fp8 = mybir.dt.float8e4
    AF = mybir.ActivationFunctionType
    OP = mybir.AluOpType
    DR = mybir.MatmulPerfMode.DoubleRow

    nc = bacc.Bacc(
        "TRN2", target_bir_lowering=False, debug=debug, num_devices=NCORES
    )

    # chunk-major operand layout: 16 column chunks of 512, each stored
    # [128 partition, 4 k, 512 col] so a chunk DMA moves 2 KB contiguous
    # per partition (128 descriptors/chunk instead of 512 -- the DGE
    # rings are descriptor-rate-bound, not bandwidth-bound, below ~2 KB)
    zt_d = nc.dram_tensor("zt", [16, P, D // P, 512], fp8, kind="ExternalInput").ap()
    zm_d = nc.dram_tensor("zm", [P, D // P, P], fp8, kind="ExternalInput").ap()
    dsum_d = nc.dram_tensor("dsum", [P, ACOLS], f32, kind="ExternalOutput").ap()

    with (
        tile.TileContext(nc) as tc,
        tc.tile_pool(name="persist", bufs=1) as persist,
        tc.tile_pool(name="expsb", bufs=4) as expsb,
        tc.tile_pool(name="mmps", bufs=2, space="PSUM") as mmps,
    ):
        zt8 = persist.tile([P, 16, D // P, 512], fp8, name="zt8", tag="zt8")
        zm8 = persist.tile([P, D // P, P], fp8, name="zm8", tag="zm8")
        acc = persist.tile([P, ACOLS], f32, name="acc", tag="acc")

        # Load the fp8 operand array.  Measured ring rates: scalar HWDGE
        # ~165 GB/s (but its engine queue blocks past 4 outstanding
        # issues, delaying queued activations); gpsimd SWDGE ~145 GB/s;
        # sync HWDGE trickles pathologically -- avoid for bulk.  The
        # s-outer sweep below needs cols [0,2048) almost immediately but
        # cols [4096,8192) only after ~40 us, so scalar takes the early
        # half (4 issues) and gpsimd the late half.
        def ld(eng, c):
            eng.dma_start(out=zt8[:, c], in_=zt_d[c])

        # interleave the early chunks across the two rings so the s=0
        # sweep's columns stream in at the finest granularity; the scalar
        # engine queue blocks past 4 outstanding issues, so it gets
        # exactly 4 and the idle gpsimd queue carries the rest via SWDGE
        # 65 KB mini-chunk (cols 0-128: the first stationary + first
        # moving block) heads the scalar ring so the exp stream can
        # start ~2 us before chunk 0 completes
        nc.scalar.dma_start(out=zm8, in_=zm_d)
        for c in (0, 2, 4):
            ld(nc.scalar, c)
        for c in (1, 3, 5, 6, 7, 8, 9, 10, 11, 12, 13, 14, 15):
            ld(nc.gpsimd, c)

        def mm_blk(ps, t, lo, hi):
            """one <=512-col output block (2 DoubleRow K passes); [lo,hi)
            must lie within a single 512-col chunk."""
            c, o0, o1 = lo // 512, lo % 512, (hi - 1) % 512 + 1
            ct, offt = (t * P) // 512, (t * P) % 512
            hp = lo % SW
            for kp in range(KP):
                nc.tensor.matmul(
                    ps[:, hp : hp + (hi - lo)],
                    zt8[:, ct, 2 * kp : 2 * kp + 2, offt : offt + P],
                    zt8[:, c, 2 * kp : 2 * kp + 2, o0:o1],
                    start=(kp == 0),
                    stop=(kp == KP - 1),
                    perf_mode=DR,
                )

        def mm_mini(ps):
            """t=0 x cols [0,128) entirely from the mini-chunk."""
            for kp in range(KP):
                nc.tensor.matmul(
                    ps[:, 0:P],
                    zm8[:, 2 * kp : 2 * kp + 2, :],
                    zm8[:, 2 * kp : 2 * kp + 2, :],
                    start=(kp == 0),
                    stop=(kp == KP - 1),
                    perf_mode=DR,
                )

        acol = 0

        def expacc(ps, lo, hi, offload=False):
            """exp + row-sum of one psum chunk.  offload=True routes the
            exp through an SBUF bf16 tile and lets the otherwise-idle
            Vector engine do the reduction, removing the ~0.2 us
            READ_ACCUMULATOR from the saturated ScalarE queue.  The psum
            tile is freed by the ACT read either way."""
            nonlocal acol
            if offload:
                sb = expsb.tile([P, SW], bf16, tag="esb", name=f"esb{acol}")
                nc.scalar.activation(
                    out=sb[:, : hi - lo],
                    in_=ps[:, lo:hi],
                    func=AF.Exp,
                    scale=float(INV_T / (SC * SC)),
                )
                nc.vector.tensor_reduce(
                    out=acc[:, acol : acol + 1],
                    in_=sb[:, : hi - lo],
                    axis=mybir.AxisListType.X,
                    op=OP.add,
                )
            else:
                nc.scalar.activation(
                    out=ps[:, lo:hi],
                    in_=ps[:, lo:hi],
                    func=AF.Exp,
                    scale=float(INV_T / (SC * SC)),
                    accum_out=acc[:, acol : acol + 1],
                )
            acol += 1

        # Each chunk gets its own psum pool tile: Tile tracks a tile as
        # one unit, so a partial ACT on a shared tile would serialize
        # against later fills into it.
        nchunk = 0

        def chunk(t, lo, hi):
            # offload full-width chunks to the DVE reduction path, except
            # every 6th (DVE reduce is ~2.2 us vs ACT ~1.9 -- 5:6 keeps
            # DVE just ahead) and the final chunks (ACT-accum keeps the
            # tail short: the output DMA would otherwise wait on a
            # trailing DVE reduce)
            nonlocal nchunk
            off = (hi - lo) == SW and nchunk % 6 != 5 and nchunk < 33
            nchunk += 1
            ps = mmps.tile([P, SW], f32, tag="ps", name=f"ps{t}_{lo}")
            if t == 0 and lo == 0 and hi == P:
                mm_mini(ps)
            else:
                b = lo
                while b < hi:
                    e = min((b // 512 + 1) * 512, hi)
                    mm_blk(ps, t, b, e)
                    b = e
            expacc(ps, lo % SW, lo % SW + (hi - lo), offload=off)

        # Ramp: pieces ordered breadth-first over row-blocks so the exp
        # stream only touches columns the DMA rings have already
        # delivered (cols [0,1024) arrive ~2 us before [1024,2048)).
        # The s-outer sweep needs only cols [0,2048) for its first 8
        # chunks, relaxing the DMA deadline for high columns to ~40 us.
        chunk(0, 0, P)
        chunk(0, P, 512)
        chunk(0, 512, 1024)
        chunk(1, 0, 1024)
        chunk(0, 1024, 2048)
        chunk(1, 1024, 2048)
        for t in range(2, MT):
            chunk(t, 0, SW)
        for s in range(1, NSW):
            for t in range(MT):
                if s == NSW - 1 and t == MT - 1:
                    # split the last chunk so the kernel does not end on
                    # a full-width activate (shorter tail)
                    chunk(t, s * SW, s * SW + 1024)
                    chunk(t, s * SW + 1024, (s + 1) * SW)
                else:
                    chunk(t, s * SW, (s + 1) * SW)

        # output on the scalar HWDGE ring -- the sync ring adds ~3 us
        # of delivery latency even for 17 KB
        nc.scalar.dma_start(out=dsum_d, in_=acc)

    nc.compile()
    return nc


def _get_nc():
    if "nc" not in _CACHE:
        _CACHE["nc"] = build()
    return _CACHE["nc"]


def _prep_host(emb_i, emb_j):
    """Normalize, scale, transpose, cast fp8; return (zt8_full, z, pos)."""
    import ml_dtypes

    z = np.concatenate(
        [np.asarray(emb_i, dtype=np.float32), np.asarray(emb_j, dtype=np.float32)],
        axis=0,
    )
    nrm = np.maximum(np.sqrt((z * z).sum(axis=1)), 1e-12)
    z /= nrm[:, None]
    pos = (z[:N] * z[N:]).sum(axis=1, dtype=np.float64)   # [N]
    zt8 = (SC * z.T).astype(ml_dtypes.float8_e4m3)        # [512, 8192]
    return zt8, pos


def _chunk_major(zt8_rot):
    """[512, 8192] -> [16, 128, 4, 512] chunk-major operand layout."""
    return np.ascontiguousarray(
        zt8_rot.reshape(D // P, P, 16, 512).transpose(2, 1, 0, 3)
    )


def make_in_maps(emb_i, emb_j):
    zt8, pos = _prep_host(emb_i, emb_j)
    _CACHE["pos"] = pos
    in_maps = []
    for c in range(NCORES):
        rot = np.roll(zt8, -c * ROWS, axis=1)
        zm = np.ascontiguousarray(
            rot[:, :P].reshape(D // P, P, P).transpose(1, 0, 2)
        )
        in_maps.append({"zt": _chunk_major(rot), "zm": zm})
    return in_maps


def finish_host(results):
    """Assemble per-core row denominators into the scalar loss."""
    denom = np.empty(N2, dtype=np.float64)
    for c in range(NCORES):
        d = results[c]["dsum"].astype(np.float64)          # [128, ACOLS]
        # row (t*128 + p) local = global c*1024 + t*128 + p
        rows = np.zeros((P, MT))
        for col, t in enumerate(ACC_T):
            rows[:, t] += d[:, col]
        denom[c * ROWS : (c + 1) * ROWS] = rows.T.reshape(ROWS)
    denom -= np.exp(INV_T)                                 # drop diagonal term
    pos = _CACHE["pos"]
    loss = np.log(denom) - INV_T * np.concatenate([pos, pos])
    return np.float32(loss.sum() / N2)


def kernel(emb_i, emb_j):
    from concourse.bass_utils import run_bass_kernel_spmd

    nc = _get_nc()
    in_maps = make_in_maps(np.asarray(emb_i), np.asarray(emb_j))
    try:
        res = run_bass_kernel_spmd(nc, in_maps, core_ids=list(range(NCORES)))
    except Exception:
        res = run_bass_kernel_spmd(nc, in_maps, core_ids=list(range(NCORES)))
    _CACHE["last_results"] = res
    return finish_host(res.results)


# revision 36
# speedup vs baseline: 1.0034x; 1.0034x over previous
"""Trainium2 Bass kernel for NT-Xent / SimCLR contrastive loss, v3.

Design (8 cores, data-parallel over rows of z = concat(z_i, z_j)):
  Host pre-normalizes z (L2 rows), scales by 8, transposes to feature-
  major [512, 8192], casts to fp8e4, and rotates by c*1024 columns per
  core so every core's own rows sit at columns [0, 1024).

  Device per core (pure matmul + exp pipeline; no transposes, casts, or
  cross-core reduction):
    - DMA the fp8 operand array (4.2 MB) in 16 chunk-major column
      chunks [128, 4, 512] (2 KB contiguous per partition -> 128
      descriptors per chunk; the DGE rings are descriptor-rate-bound
      for smaller runs), early chunks interleaved across the scalar
      HWDGE + gpsimd SWDGE rings.
    - Sweep (s-outer, t-inner): stationary = my row-block t (fp8
      DoubleRow, K=512 in 2 passes), moving = column chunk s of 2048 ->
      psum [128, 2048] = 64*sim; the ramp is split into pieces ordered
      breadth-first over row-blocks so the exp stream starts ~11 us in
      and only touches delivered columns.
    - ScalarE exp(0.03125 * psum) -> per-row partial sums of
      exp(2*sim[r, :]); most chunks route the exp through an SBUF bf16
      tile so the idle Vector engine does the row-sum off the critical
      path, keeping ScalarE's queue free of accumulator reads; 36
      partials in acc[128, 36].
  The ScalarE exp stream is the bottleneck and runs near-gap-free to
  the end; matmuls (~64 us) and DVE reductions (~52 us) hide behind
  it.
  Host: denom[r] = sum acc cols per t - exp(2); positives from fp32 z;
  loss = mean(log(denom) - 2*pos).
"""

import sys

if "/opt/trn_rl_repo" not in sys.path:
    sys.path.insert(0, "/opt/trn_rl_repo")

import numpy as np

N = 4096
D = 512
TEMP = 0.5
INV_T = 1.0 / TEMP

N2 = 2 * N            # 8192
NCORES = 8
ROWS = N2 // NCORES   # 1024 rows per core
P = 128
MT = ROWS // P        # 8 stationary row-blocks per core
SW = 2048             # moving sweep chunk (4 psum banks)
NSW = N2 // SW        # 4 sweep chunks
KP = 2                # DoubleRow K passes (256 features each)
SC = 8.0              # fp8 operand scale; psum = SC*SC*sim
# accumulator columns in device emission order: ramp pieces
# (t0/t0/t1/t0/t1), s=0 remainder, then s=1..3 with the last chunk split
ACC_T = (
    [0, 0, 1, 0, 1]
    + list(range(2, MT))
    + [_t for _s in range(1, NSW) for _t in range(MT)]
    + [MT - 1]
)
ACOLS = len(ACC_T)   # 36

_CACHE = {}


def build(debug=False):
    import concourse.bacc as bacc
    import concourse.tile as tile
    from concourse import mybir

    f32 = mybir.dt.float32
    bf16 = mybir.dt.bfloat16
    fp8 = mybir.dt.float8e4
    AF = mybir.ActivationFunctionType
    OP = mybir.AluOpType
    DR = mybir.MatmulPerfMode.DoubleRow

    nc = bacc.Bacc(
        "TRN2", target_bir_lowering=False, debug=debug, num_devices=NCORES
    )

    # chunk-major operand layout: 16 column chunks of 512, each stored
    # [128 partition, 4 k, 512 col] so a chunk DMA moves 2 KB contiguous
    # per partition (128 descriptors/chunk instead of 512 -- the DGE
    # rings are descriptor-rate-bound, not bandwidth-bound, below ~2 KB)
    zt_d = nc.dram_tensor("zt", [16, P, D // P, 512], fp8, kind="ExternalInput").ap()
    dsum_d = nc.dram_tensor("dsum", [P, ACOLS], f32, kind="ExternalOutput").ap()

    with (
        tile.TileContext(nc) as tc,
        tc.tile_pool(name="persist", bufs=1) as persist,
        tc.tile_pool(name="expsb", bufs=4) as expsb,
        tc.tile_pool(name="mmps", bufs=2, space="PSUM") as mmps,
    ):
        zt8 = persist.tile([P, 16, D // P, 512], fp8, name="zt8", tag="zt8")
        acc = persist.tile([P, ACOLS], f32, name="acc", tag="acc")

        # Load the fp8 operand array.  Measured ring rates: scalar HWDGE
        # ~165 GB/s (but its engine queue blocks past 4 outstanding
        # issues, delaying queued activations); gpsimd SWDGE ~145 GB/s;
        # sync HWDGE trickles pathologically -- avoid for bulk.  The
        # s-outer sweep below needs cols [0,2048) almost immediately but
        # cols [4096,8192) only after ~40 us, so scalar takes the early
        # half (4 issues) and gpsimd the late half.
        def ld(eng, c):
            eng.dma_start(out=zt8[:, c], in_=zt_d[c])

        # interleave the early chunks across the two rings so the s=0
        # sweep's columns stream in at the finest granularity; the scalar
        # engine queue blocks past 4 outstanding issues, so it gets
        # exactly 4 and the idle gpsimd queue carries the rest via SWDGE
        for c in (0, 2, 4, 6):
            ld(nc.scalar, c)
        for c in (1, 3, 5, 7, 8, 9, 10, 11, 12, 13, 14, 15):
            ld(nc.gpsimd, c)

        def mm_512(ps, t, h512):
            """one 512-col output block (2 DoubleRow K passes)."""
            m0 = h512 * 512
            ct, offt = (t * P) // 512, (t * P) % 512
            hp = (h512 % (SW // 512)) * 512
            for kp in range(KP):
                nc.tensor.matmul(
                    ps[:, hp : hp + 512],
                    zt8[:, ct, 2 * kp : 2 * kp + 2, offt : offt + P],
                    zt8[:, h512, 2 * kp : 2 * kp + 2, :],
                    start=(kp == 0),
                    stop=(kp == KP - 1),
                    perf_mode=DR,
                )

        acol = 0

        def expacc(ps, lo, hi, offload=False):
            """exp + row-sum of one psum chunk.  offload=True routes the
            exp through an SBUF bf16 tile and lets the otherwise-idle
            Vector engine do the reduction, removing the ~0.2 us
            READ_ACCUMULATOR from the saturated ScalarE queue.  The psum
            tile is freed by the ACT read either way."""
            nonlocal acol
            if offload:
                sb = expsb.tile([P, SW], bf16, tag="esb", name=f"esb{acol}")
                nc.scalar.activation(
                    out=sb[:, : hi - lo],
                    in_=ps[:, lo:hi],
                    func=AF.Exp,
                    scale=float(INV_T / (SC * SC)),
                )
                nc.vector.tensor_reduce(
                    out=acc[:, acol : acol + 1],
                    in_=sb[:, : hi - lo],
                    axis=mybir.AxisListType.X,
                    op=OP.add,
                )
            else:
                nc.scalar.activation(
                    out=ps[:, lo:hi],
                    in_=ps[:, lo:hi],
                    func=AF.Exp,
                    scale=float(INV_T / (SC * SC)),
                    accum_out=acc[:, acol : acol + 1],
                )
            acol += 1

        # Each chunk gets its own psum pool tile: Tile tracks a tile as
        # one unit, so a partial ACT on a shared tile would serialize
        # against later fills into it.
        nchunk = 0

        def chunk(t, lo, hi):
            # offload full-width chunks to the DVE reduction path, except
            # every 6th (DVE reduce is ~2.2 us vs ACT ~1.9 -- 5:6 keeps
            # DVE just ahead) and the final chunks (ACT-accum keeps the
            # tail short: the output DMA would otherwise wait on a
            # trailing DVE reduce)
            nonlocal nchunk
            off = (hi - lo) == SW and nchunk % 6 != 5 and nchunk < 33
            nchunk += 1
            ps = mmps.tile([P, SW], f32, tag="ps", name=f"ps{t}_{lo}")
            for h in range(lo // 512, hi // 512):
                mm_512(ps, t, h)
            expacc(ps, lo % SW, lo % SW + (hi - lo), offload=off)

        # Ramp: pieces ordered breadth-first over row-blocks so the exp
        # stream only touches columns the DMA rings have already
        # delivered (cols [0,1024) arrive ~2 us before [1024,2048)).
        # The s-outer sweep needs only cols [0,2048) for its first 8
        # chunks, relaxing the DMA deadline for high columns to ~40 us.
        chunk(0, 0, 512)
        chunk(0, 512, 1024)
        chunk(1, 0, 1024)
        chunk(0, 1024, 2048)
        chunk(1, 1024, 2048)
        for t in range(2, MT):
            chunk(t, 0, SW)
        for s in range(1, NSW):
            for t in range(MT):
                if s == NSW - 1 and t == MT - 1:
                    # split the last chunk so the kernel does not end on
                    # a full-width activate (shorter tail)
                    chunk(t, s * SW, s * SW + 1024)
                    chunk(t, s * SW + 1024, (s + 1) * SW)
                else:
                    chunk(t, s * SW, (s + 1) * SW)

        # output on the scalar HWDGE ring -- the sync ring adds ~3 us
        # of delivery latency even for 17 KB
        nc.scalar.dma_start(out=dsum_d, in_=acc)

    nc.compile()
    return nc


def _get_nc():
    if "nc" not in _CACHE:
        _CACHE["nc"] = build()
    return _CACHE["nc"]


def _prep_host(emb_i, emb_j):
    """Normalize, scale, transpose, cast fp8; return (zt8_full, z, pos)."""
    import ml_dtypes

    z = np.concatenate(
        [np.asarray(emb_i, dtype=np.float32), np.asarray(emb_j, dtype=np.float32)],
        axis=0,
    )
    nrm = np.maximum(np.sqrt((z * z).sum(axis=1)), 1e-12)
    z /= nrm[:, None]
    pos = (z[:N] * z[N:]).sum(axis=1, dtype=np.float64)   # [N]
    zt8 = (SC * z.T).astype(ml_dtypes.float8_e4m3)        # [512, 8192]
    return zt8, pos


def _chunk_major(zt8_rot):
    """[512, 8192] -> [16, 128, 4, 512] chunk-major operand layout."""
    return np.ascontiguousarray(
        zt8_rot.reshape(D // P, P, 16, 512).transpose(2, 1, 0, 3)
    )


def make_in_maps(emb_i, emb_j):
    zt8, pos = _prep_host(emb_i, emb_j)
    _CACHE["pos"] = pos
    in_maps = []
    for c in range(NCORES):
        rot = np.roll(zt8, -c * ROWS, axis=1)
        in_maps.append({"zt": _chunk_major(rot)})
    return in_maps


def finish_host(results):
    """Assemble per-core row denominators into the scalar loss."""
    denom = np.empty(N2, dtype=np.float64)
    for c in range(NCORES):
        d = results[c]["dsum"].astype(np.float64)          # [128, ACOLS]
        # row (t*128 + p) local = global c*1024 + t*128 + p
        rows = np.zeros((P, MT))
        for col, t in enumerate(ACC_T):
            rows[:, t] += d[:, col]
        denom[c * ROWS : (c + 1) * ROWS] = rows.T.reshape(ROWS)
    denom -= np.exp(INV_T)                                 # drop diagonal term
    pos = _CACHE["pos"]
    loss = np.log(denom) - INV_T * np.concatenate([pos, pos])
    return np.float32(loss.sum() / N2)


def kernel(emb_i, emb_j):
    from concourse.bass_utils import run_bass_kernel_spmd

    nc = _get_nc()
    in_maps = make_in_maps(np.asarray(emb_i), np.asarray(emb_j))
    try:
        res = run_bass_kernel_spmd(nc, in_maps, core_ids=list(range(NCORES)))
    except Exception:
        res = run_bass_kernel_spmd(nc, in_maps, core_ids=list(range(NCORES)))
    _CACHE["last_results"] = res
    return finish_host(res.results)
